# revision 3
# baseline (speedup 1.0000x reference)
"""Causal self-attention Trainium2 kernel v2 (B=4, S=2048, D=1024, H=16, Dh=64).

Sharding: 8 cores = 4 batches x 2 head-groups (8 heads each). Each core
computes the qkv projection for its heads, causal attention, and a partial
output projection; the host sums the two partials per batch and adds b_out.

v2 changes vs baseline:
  - fp16 storage everywhere (x, W, qT/kT, v, eT, outT, DRAM out); PSUM stays
    fp32. fp16 matmuls run 1 PE cycle/row at ANY free size (fp32r needs
    N>=256), so diagonal blocks use exact widths (512/384/256/128).
  - weights loaded once and SBUF-resident (baseline re-DMA'd wq/wk per
    chunk: 26MB -> ~11MB per-core DMA per iteration).
  - fused pipeline: per query-chunk qc emit A(qc) projection, B(qc)
    attention, C1(qc) normalize, C2(qc) out-projection. The tile scheduler
    overlaps A(qc+1) PE work under B(qc)'s ACT(exp) latency.
  - QK for the two heads of a pair issue back-to-back as K=64 matmuls on
    PE row-tiles (0,0)/(64,0) (64x128 tiling mode) so they run
    concurrently; causal masks are applied as two more K=64 row-tiled
    matmuls (ident slices) in the same mode. Only PV (K=128) switches mode.
  - exp for both heads in one ACT instruction over the 2-bank qk tile.
  - denominators via the ones-column-in-v trick (as baseline); rc tile is
    memset once per pair, reciprocal applied per chunk slice.
"""
import os
import numpy as np
from contextlib import ExitStack

GPSIMD_DMA = os.environ.get("KV2_GPSIMD_DMA", "1") == "1"
EXP_SPAN = os.environ.get("KV2_EXP_SPAN", "1") == "1"
PHASES = os.environ.get("KV2_PHASES", "abc")
B_PARTS = os.environ.get("KV2_B_PARTS", "qmep")  # q=QK m=mask e=exp p=PV
SKIP_A = os.environ.get("KV2_SKIP_A", "0") == "1"
APPROX_RECIP = os.environ.get("KV2_APPROX_RECIP", "1") == "1"
MASK_DVE = os.environ.get("KV2_MASK_DVE", "1") == "1"

import concourse.bass as bass
import concourse.tile as tile
from concourse import bacc, mybir
from concourse.bass_utils import run_bass_kernel_spmd

P = 128
D_MODEL = 1024
NHEAD = 16
HEAD_DIM = 64
B = 4
S = 2048
N_CORES = 8
HEADS_LOC = 8           # heads per core
NPAIR = HEADS_LOC // 2  # head pairs per core
DL = HEADS_LOC * HEAD_DIM  # local dims = 512
NDT = D_MODEL // P      # 8 d-tiles
NQC = S // 512          # sq chunks of 512
NST = S // P            # s tiles of 128
VSLOT = 192             # per-pair v slot: v_h0(64) | ones(64) | v_h1(64)
F32 = mybir.dt.float32
F32R = mybir.dt.float32r
F16 = mybir.dt.float16
SCALE = HEAD_DIM ** -0.5

_NC_CACHE = {}


def build_nc(loop=None, loop_n=17):
    key = (loop, loop_n if loop else 0)
    if key in _NC_CACHE:
        return _NC_CACHE[key]
    nc = bacc.Bacc("TRN2", target_bir_lowering=False, debug=False,
                   num_devices=N_CORES)
    xT = nc.dram_tensor("xT", [D_MODEL, S], F16, kind="ExternalInput").ap()
    wq = nc.dram_tensor("wq", [D_MODEL, DL], F16, kind="ExternalInput").ap()
    wk = nc.dram_tensor("wk", [D_MODEL, DL], F16, kind="ExternalInput").ap()
    wv = nc.dram_tensor("wv", [D_MODEL, DL], F16, kind="ExternalInput").ap()
    wout = nc.dram_tensor("wout", [DL, D_MODEL], F16, kind="ExternalInput").ap()
    bq = nc.dram_tensor("bq", [P, NPAIR], F32, kind="ExternalInput").ap()
    bk = nc.dram_tensor("bk", [P, NPAIR], F32, kind="ExternalInput").ap()
    bv = nc.dram_tensor("bv", [P, NPAIR], F32, kind="ExternalInput").ap()
    ident = nc.dram_tensor("ident", [P, P], F16, kind="ExternalInput").ap()
    negmask = nc.dram_tensor("negmask", [P, P], F16, kind="ExternalInput").ap()
    binmask = nc.dram_tensor("binmask", [P, P], F16, kind="ExternalInput").ap()
    sel = nc.dram_tensor("sel", [65, P], F32R, kind="ExternalInput").ap()
    onesv = nc.dram_tensor("onesv", [P, 4, NPAIR, 64], F16,
                           kind="ExternalInput").ap()
    out = nc.dram_tensor("out", [S, D_MODEL], F16, kind="ExternalOutput").ap()

    wqr = wq.rearrange("(a p) n -> p a n", p=P)
    wkr = wk.rearrange("(a p) n -> p a n", p=P)
    wvr = wv.rearrange("(a p) n -> p a n", p=P)
    woutr = wout.rearrange("(a p) (b n) -> p a b n", p=P, n=512)
    xTr = xT.rearrange("(a p) s -> p a s", p=P)

    with tile.TileContext(nc) as tc, ExitStack() as ctx, \
         nc.allow_low_precision(reason="fp16 storage is within tolerance"):
        # ---- persistent SBUF ----
        persist = ctx.enter_context(tc.tile_pool(name="persist", bufs=1))
        qTt = [[persist.tile([P, 512], F16, tag=f"qT{p}_{qc}",
                             name=f"qT{p}_{qc}")
                for qc in range(NQC)] for p in range(NPAIR)]
        kTt = [[persist.tile([P, 512], F16, tag=f"kT{p}_{qc}",
                             name=f"kT{p}_{qc}")
                for qc in range(NQC)] for p in range(NPAIR)]
        vaugt = [persist.tile([P, 4, NPAIR, VSLOT], F16, tag=f"v{qc}",
                              name=f"vaug{qc}")
                 for qc in range(NQC)]
        outT = persist.tile([P, NPAIR, S], F16, tag="outT", name="outT")
        rct = [persist.tile([65, NQC, 512], F32R, tag=f"rc{pp}",
                            name=f"rc{pp}") for pp in range(NPAIR)]

        consts = ctx.enter_context(tc.tile_pool(name="consts", bufs=1))
        wq_sb = consts.tile([P, NDT, DL], F16, tag="wq")
        wk_sb = consts.tile([P, NDT, DL], F16, tag="wk")
        wv_sb = consts.tile([P, NDT, DL], F16, tag="wv")
        wout_sb = consts.tile([P, NPAIR, 2, 512], F16, tag="wout")
        bq_sb = consts.tile([P, NPAIR], F32, tag="bq")
        bk_sb = consts.tile([P, NPAIR], F32, tag="bk")
        bv_sb = consts.tile([P, NPAIR], F32, tag="bv")
        id_sb = consts.tile([P, P], F16, tag="ident")
        nm_sb = consts.tile([P, P], F16, tag="negmask")
        bm_sb = consts.tile([P, P], F16, tag="binmask")
        sel_sb = consts.tile([65, P], F32R, tag="sel")
        # wq on the sync queue ahead of the x loads so A(0)'s first matmuls
        # start early; everything else one-time goes via the gpsimd SWDGE
        # queue so it streams in parallel with the sync queue.
        nc.sync.dma_start(wq_sb, wqr)
        nc.sync.dma_start(bq_sb, bq)
        dmae = nc.gpsimd if GPSIMD_DMA else nc.sync
        dmae.dma_start(wk_sb, wkr)
        dmae.dma_start(bk_sb, bk)
        dmae.dma_start(wv_sb, wvr)
        dmae.dma_start(bv_sb, bv)
        dmae.dma_start(id_sb, ident)
        dmae.dma_start(nm_sb, negmask)
        dmae.dma_start(bm_sb, binmask)
        dmae.dma_start(sel_sb, sel)
        for qc in range(NQC):
            dmae.dma_start(vaugt[qc][:, :, :, 64:128], onesv)
        dmae.dma_start(wout_sb, woutr)

        xts = {}

        def phase_a_piece(qc, p, xtp, psp):
            """1/4 of chunk qc's projection: qT/kT for pair p, v for st4=p."""
            if qc not in xts:
                cw = slice(qc * 512, (qc + 1) * 512)
                xt = xtp.tile([P, NDT, 512], F16, tag="xt")
                nc.sync.dma_start(xt, xTr[:, :, cw])
                xts[qc] = xt
            xt = xts[qc]
            psq = psp.tile([P, 512], F32, tag="ps", name=f"psq{p}")
            for dt in range(NDT):
                nc.tensor.matmul(psq, wq_sb[:, dt, bass.ts(p, P)],
                                 xt[:, dt],
                                 start=(dt == 0), stop=(dt == NDT - 1),
                                 skip_group_check=SKIP_A)
            nc.vector.tensor_scalar_add(qTt[p][qc], psq, bq_sb[:, p:p + 1])
            psk = psp.tile([P, 512], F32, tag="ps", name=f"psk{p}")
            for dt in range(NDT):
                nc.tensor.matmul(psk, wk_sb[:, dt, bass.ts(p, P)],
                                 xt[:, dt],
                                 start=(dt == 0), stop=(dt == NDT - 1),
                                 skip_group_check=SKIP_A)
            nc.vector.tensor_scalar_add(kTt[p][qc], psk, bk_sb[:, p:p + 1])
            st4 = p
            psv = psp.tile([P, 512], F32, tag="ps", name=f"psv{st4}")
            for dt in range(NDT):
                nc.tensor.matmul(psv, xt[:, dt, bass.ts(st4, P)],
                                 wv_sb[:, dt],
                                 start=(dt == 0), stop=(dt == NDT - 1),
                                 skip_group_check=SKIP_A)
            psv4 = psv.rearrange("q (pp hh d) -> q pp hh d", pp=NPAIR, hh=2)
            nc.vector.tensor_copy(vaugt[qc][:, st4, :, 0:64],
                                  psv4[:, :, 0, :])
            nc.vector.tensor_copy(vaugt[qc][:, st4, :, 128:192],
                                  psv4[:, :, 1, :])

        def phase_b(qc, pp, eTp, qkp, pvp):
            """attention for (chunk qc, pair pp): blocks j=0..4qc+3."""
            cw = slice(qc * 512, (qc + 1) * 512)
            pv0 = pvp.tile([65, 512], F32, tag="pv0", name="pv0")
            pv1 = pvp.tile([P, 512], F32, tag="pv1", name="pv1")
            nblk = 4 * qc + 4
            for j in range(nblk):
                c0 = max(0, (j - 4 * qc) * P)
                diag = j >= 4 * qc
                qk = qkp.tile([P, 2, 512], F32, tag="qk", name="qk")
                # two K=64 QK matmuls back-to-back on PE row-tiles
                # (0,0)/(64,0): they execute concurrently in 64x128 mode.
                mask_on = diag and "m" in B_PARTS and not MASK_DVE
                if "q" in B_PARTS:
                    for h in range(2):
                        hw = slice(64 * h, 64 * h + 64)
                        nc.tensor.matmul(
                            qk[:, h, c0:],
                            kTt[pp][j // 4][hw, (j % 4) * P:(j % 4 + 1) * P],
                            qTt[pp][qc][hw, c0:],
                            start=True, stop=not mask_on,
                            skip_group_check=True)
                if mask_on:
                    # causal mask add (-30000 where sk>sq): one K=128 matmul
                    # per head (two K=64 row-tiles would race on the bank).
                    for h in range(2):
                        nc.tensor.matmul(
                            qk[:, h, c0:c0 + P], id_sb, nm_sb,
                            start=False, stop=True,
                            skip_group_check=True)
                eT = eTp.tile([P, 2, 512], F16, tag="eT", name="eT")
                if "e" in B_PARTS:
                    if EXP_SPAN:
                        nc.scalar.activation(
                            eT[:, :, c0:], qk[:, :, c0:],
                            mybir.ActivationFunctionType.Exp, scale=SCALE)
                    else:
                        for h in range(2):
                            nc.scalar.activation(
                                eT[:, h, c0:], qk[:, h, c0:],
                                mybir.ActivationFunctionType.Exp, scale=SCALE)
                else:
                    nc.vector.memset(eT.bitcast(F32), 1.0)
                if diag and "m" in B_PARTS and MASK_DVE:
                    # zero the causal triangle of eT post-exp on DVE
                    # (keeps the mask off the LDW-serialized PE).
                    for h in range(2):
                        nc.vector.tensor_mul(eT[:, h, c0:c0 + P],
                                             eT[:, h, c0:c0 + P], bm_sb)
                if "p" in B_PARTS:
                    vj = vaugt[j // 4][:, j % 4, pp]
                    nc.tensor.matmul(pv0[:, c0:], vj[:, 0:65], eT[:, 0, c0:],
                                     start=(j == 0), stop=(j == nblk - 1),
                                     skip_group_check=True)
                    nc.tensor.matmul(pv1[:, c0:], vj[:, 64:192], eT[:, 1, c0:],
                                     start=(j == 0), stop=(j == nblk - 1),
                                     skip_group_check=True)
            nc.vector.tensor_copy(outT[0:64, pp, cw], pv0[0:64])
            nc.vector.tensor_copy(outT[64:128, pp, cw], pv1[64:128])
            nc.vector.tensor_copy(rct[pp][64:65, qc], pv0[64:65])
            nc.vector.tensor_copy(rct[pp][0:1, qc], pv1[0:1])

        def phase_c1(qc, pp, qkp, scrp):
            """normalize chunk qc of pair pp by 1/den, add bv."""
            cw = slice(qc * 512, (qc + 1) * 512)
            rc = rct[pp]
            # approx reciprocal (~51 ULP, 5x faster than the iterative
            # divide); fp32 scratch + rounding copy keeps the fp32r
            # verifier happy on the bc-matmul operand.
            if APPROX_RECIP:
                scr = scrp.tile([65, 512], F32, tag="scr", name="scr")
                nc.vector.reciprocal_approx_fast(scr, rc[:, qc].bitcast(F32))
                nc.vector.tensor_copy(rc[:, qc], scr)
            else:
                nc.vector.reciprocal(rc[:, qc], rc[:, qc])
            bc = qkp.tile([P, 2, 512], F32, tag="qk", name="bc")[:, 0]
            nc.tensor.matmul(bc, sel_sb, rc[:, qc], start=True, stop=True)
            nc.vector.tensor_mul(outT[:, pp, cw], outT[:, pp, cw], bc)
            nc.vector.tensor_scalar_add(outT[:, pp, cw], outT[:, pp, cw],
                                        bv_sb[:, pp:pp + 1])

        def phase_c2(qc, outp, psp):
            """output projection for the 4 seq-tiles of chunk qc."""
            for st in range(4 * qc, 4 * qc + 4):
                sw = slice(st * P, (st + 1) * P)
                for dc in range(2):
                    ps = psp.tile([P, 512], F32, tag="ps", name="psC")
                    for p in range(NPAIR):
                        nc.tensor.matmul(ps, outT[:, p, sw],
                                         wout_sb[:, p, dc],
                                         start=(p == 0), stop=(p == NPAIR - 1),
                                         skip_group_check=SKIP_A)
                    ot = outp.tile([P, 512], F16, tag="ot")
                    nc.vector.tensor_copy(ot, ps)
                    nc.sync.dma_start(out[sw, dc * 512:(dc + 1) * 512], ot)

        # rc rows 1..63 are read (x0-weighted) by the bc matmul; memset once
        # so the PE never multiplies uninitialized SBUF (they stay ~1.0).
        for pp in range(NPAIR):
            nc.vector.memset(rct[pp].bitcast(F32), 1.0)

        def body(xtp, psp, eTp, qkp, pvp, outp, scrp):
            xts.clear()
            for pp in range(NPAIR):
                phase_a_piece(0, pp, xtp, psp)
            for qc in range(NQC):
                # emit next chunk's projection pieces between this chunk's
                # attention pair-streams so the scheduler has PE work to
                # fill B's exp-latency bubbles (pool buffers recycle in
                # emission order, so interleaved emission is required).
                for pp in range(NPAIR):
                    if "b" in PHASES:
                        phase_b(qc, pp, eTp, qkp, pvp)
                    if qc + 1 < NQC:
                        phase_a_piece(qc + 1, pp, xtp, psp)
                if "c" in PHASES:
                    for pp in range(NPAIR):
                        phase_c1(qc, pp, qkp, scrp)
                    phase_c2(qc, outp, psp)

        with tc.tile_pool(name="xtp", bufs=2) as xtp, \
             tc.tile_pool(name="eTp", bufs=3) as eTp, \
             tc.tile_pool(name="scrp", bufs=2) as scrp, \
             tc.tile_pool(name="outp", bufs=3) as outp, \
             tc.tile_pool(name="psp", bufs=2, space="PSUM") as psp, \
             tc.tile_pool(name="qkp", bufs=2, space="PSUM") as qkp, \
             tc.tile_pool(name="pvp", bufs=1, space="PSUM") as pvp:
            if loop == "full":
                with tc.For_i(0, loop_n, 1):
                    body(xtp, psp, eTp, qkp, pvp, outp, scrp)
            else:
                body(xtp, psp, eTp, qkp, pvp, outp, scrp)
    nc.compile()
    _NC_CACHE[key] = nc
    return nc


def prep_core_inputs(x, W_qkv, b_qkv, W_out, core):
    b, g = core // 2, core % 2
    hs = slice(HEADS_LOC * g, HEADS_LOC * (g + 1))
    w3 = W_qkv.reshape(D_MODEL, 3, NHEAD, HEAD_DIM)
    b3 = b_qkv.reshape(3, NHEAD, HEAD_DIM)
    # bc-broadcast selector: den for (chunk, h0) sits at rc row 64,
    # (chunk, h1) at rc row 0; head h covers output partitions [64h, 64h+64).
    sel = np.zeros((65, P), np.float32)
    sel[64, :64] = 1.0
    sel[0, 64:] = 1.0
    tri = np.where(np.arange(P)[:, None] > np.arange(P)[None, :],
                   np.float16(-30000.0), np.float16(0.0))
    bm = np.where(np.arange(P)[:, None] > np.arange(P)[None, :],
                  np.float16(0.0), np.float16(1.0))
    return {
        "xT": np.ascontiguousarray(x[b].T).astype(np.float16),
        "wq": w3[:, 0, hs].reshape(D_MODEL, DL).astype(np.float16),
        "wk": w3[:, 1, hs].reshape(D_MODEL, DL).astype(np.float16),
        "wv": w3[:, 2, hs].reshape(D_MODEL, DL).astype(np.float16),
        "wout": np.ascontiguousarray(
            W_out.reshape(NHEAD, HEAD_DIM, D_MODEL)[hs].reshape(
                DL, D_MODEL)).astype(np.float16),
        "bq": np.ascontiguousarray(b3[0, hs].reshape(NPAIR, P).T,
                                   dtype=np.float32),
        "bk": np.ascontiguousarray(b3[1, hs].reshape(NPAIR, P).T,
                                   dtype=np.float32),
        "bv": np.ascontiguousarray(b3[2, hs].reshape(NPAIR, P).T,
                                   dtype=np.float32),
        "ident": np.eye(P, dtype=np.float16),
        "negmask": np.ascontiguousarray(tri, dtype=np.float16),
        "binmask": np.ascontiguousarray(bm, dtype=np.float16),
        "sel": sel,
        "onesv": np.ones((P, 4, NPAIR, 64), np.float16),
    }


def kernel(x, W_qkv, b_qkv, W_out, b_out):
    x = np.asarray(x, np.float32)
    W_qkv = np.asarray(W_qkv, np.float32)
    b_qkv = np.asarray(b_qkv, np.float32)
    W_out = np.asarray(W_out, np.float32)
    b_out = np.asarray(b_out, np.float32)
    nc = build_nc()
    in_maps = [prep_core_inputs(x, W_qkv, b_qkv, W_out, c)
               for c in range(N_CORES)]
    res = run_bass_kernel_spmd(nc, in_maps, core_ids=list(range(N_CORES)))
    out = np.empty((B, S, D_MODEL), np.float32)
    for b in range(B):
        out[b] = (res.results[2 * b]["out"].astype(np.float32)
                  + res.results[2 * b + 1]["out"].astype(np.float32) + b_out)
    return out


# revision 4
# speedup vs baseline: 1.0186x; 1.0186x over previous
"""Causal self-attention Trainium2 kernel v2 (B=4, S=2048, D=1024, H=16, Dh=64).

Sharding: 8 cores = 4 batches x 2 head-groups (8 heads each). Each core
computes the qkv projection for its heads, causal attention, and a partial
output projection; the host sums the two partials per batch and adds b_out.

v2 changes vs baseline:
  - fp16 storage everywhere (x, W, qT/kT, v, eT, outT, DRAM out); PSUM stays
    fp32. fp16 matmuls run 1 PE cycle/row at ANY free size (fp32r needs
    N>=256), so diagonal blocks use exact widths (512/384/256/128).
  - weights loaded once and SBUF-resident (baseline re-DMA'd wq/wk per
    chunk: 26MB -> ~11MB per-core DMA per iteration).
  - fused pipeline: per query-chunk qc emit A(qc) projection, B(qc)
    attention, C1(qc) normalize, C2(qc) out-projection. The tile scheduler
    overlaps A(qc+1) PE work under B(qc)'s ACT(exp) latency.
  - QK for the two heads of a pair issue back-to-back as K=64 matmuls on
    PE row-tiles (0,0)/(64,0) (64x128 tiling mode) so they run
    concurrently; causal masks are applied as two more K=64 row-tiled
    matmuls (ident slices) in the same mode. Only PV (K=128) switches mode.
  - exp for both heads in one ACT instruction over the 2-bank qk tile.
  - denominators via the ones-column-in-v trick (as baseline); rc tile is
    memset once per pair, reciprocal applied per chunk slice.
"""
import os
import numpy as np
from contextlib import ExitStack

GPSIMD_DMA = os.environ.get("KV2_GPSIMD_DMA", "1") == "1"
EXP_SPAN = os.environ.get("KV2_EXP_SPAN", "1") == "1"
PHASES = os.environ.get("KV2_PHASES", "abc")
B_PARTS = os.environ.get("KV2_B_PARTS", "qmep")  # q=QK m=mask e=exp p=PV
SKIP_A = os.environ.get("KV2_SKIP_A", "0") == "1"
APPROX_RECIP = os.environ.get("KV2_APPROX_RECIP", "1") == "1"
MASK_DVE = os.environ.get("KV2_MASK_DVE", "1") == "1"

import concourse.bass as bass
import concourse.tile as tile
from concourse import bacc, mybir
from concourse.bass_utils import run_bass_kernel_spmd

P = 128
D_MODEL = 1024
NHEAD = 16
HEAD_DIM = 64
B = 4
S = 2048
N_CORES = 8
HEADS_LOC = 8           # heads per core
NPAIR = HEADS_LOC // 2  # head pairs per core
DL = HEADS_LOC * HEAD_DIM  # local dims = 512
NDT = D_MODEL // P      # 8 d-tiles
NQC = S // 512          # sq chunks of 512
NST = S // P            # s tiles of 128
VSLOT = 192             # per-pair v slot: v_h0(64) | ones(64) | v_h1(64)
F32 = mybir.dt.float32
F32R = mybir.dt.float32r
F16 = mybir.dt.float16
SCALE = HEAD_DIM ** -0.5

_NC_CACHE = {}


def build_nc(loop=None, loop_n=17):
    key = (loop, loop_n if loop else 0)
    if key in _NC_CACHE:
        return _NC_CACHE[key]
    nc = bacc.Bacc("TRN2", target_bir_lowering=False, debug=False,
                   num_devices=N_CORES)
    xT = nc.dram_tensor("xT", [D_MODEL, S], F16, kind="ExternalInput").ap()
    wq = nc.dram_tensor("wq", [D_MODEL, DL], F16, kind="ExternalInput").ap()
    wk = nc.dram_tensor("wk", [D_MODEL, DL], F16, kind="ExternalInput").ap()
    wv = nc.dram_tensor("wv", [D_MODEL, DL], F16, kind="ExternalInput").ap()
    wout = nc.dram_tensor("wout", [DL, D_MODEL], F16, kind="ExternalInput").ap()
    bq = nc.dram_tensor("bq", [P, NPAIR], F32, kind="ExternalInput").ap()
    bk = nc.dram_tensor("bk", [P, NPAIR], F32, kind="ExternalInput").ap()
    bv = nc.dram_tensor("bv", [P, NPAIR], F32, kind="ExternalInput").ap()
    ident = nc.dram_tensor("ident", [P, P], F16, kind="ExternalInput").ap()
    negmask = nc.dram_tensor("negmask", [P, P], F16, kind="ExternalInput").ap()
    binmask = nc.dram_tensor("binmask", [P, P], F16, kind="ExternalInput").ap()
    sel = nc.dram_tensor("sel", [65, P], F32R, kind="ExternalInput").ap()
    onesv = nc.dram_tensor("onesv", [P, 4, NPAIR, 64], F16,
                           kind="ExternalInput").ap()
    out = nc.dram_tensor("out", [S, D_MODEL], F16, kind="ExternalOutput").ap()

    wqr = wq.rearrange("(a p) n -> p a n", p=P)
    wkr = wk.rearrange("(a p) n -> p a n", p=P)
    wvr = wv.rearrange("(a p) n -> p a n", p=P)
    woutr = wout.rearrange("(a p) (b n) -> p a b n", p=P, n=512)
    xTr = xT.rearrange("(a p) s -> p a s", p=P)

    with tile.TileContext(nc) as tc, ExitStack() as ctx, \
         nc.allow_low_precision(reason="fp16 storage is within tolerance"):
        # ---- persistent SBUF ----
        persist = ctx.enter_context(tc.tile_pool(name="persist", bufs=1))
        qTt = [[persist.tile([P, 512], F16, tag=f"qT{p}_{qc}",
                             name=f"qT{p}_{qc}")
                for qc in range(NQC)] for p in range(NPAIR)]
        kTt = [[persist.tile([P, 512], F16, tag=f"kT{p}_{qc}",
                             name=f"kT{p}_{qc}")
                for qc in range(NQC)] for p in range(NPAIR)]
        vaugt = [persist.tile([P, 4, NPAIR, VSLOT], F16, tag=f"v{qc}",
                              name=f"vaug{qc}")
                 for qc in range(NQC)]
        outT = persist.tile([P, NPAIR, S], F16, tag="outT", name="outT")
        rct = [persist.tile([65, NQC, 512], F32R, tag=f"rc{pp}",
                            name=f"rc{pp}") for pp in range(NPAIR)]

        consts = ctx.enter_context(tc.tile_pool(name="consts", bufs=1))
        wq_sb = consts.tile([P, NDT, DL], F16, tag="wq")
        wk_sb = consts.tile([P, NDT, DL], F16, tag="wk")
        wv_sb = consts.tile([P, NDT, DL], F16, tag="wv")
        wout_sb = consts.tile([P, NPAIR, 2, 512], F16, tag="wout")
        bq_sb = consts.tile([P, NPAIR], F32, tag="bq")
        bk_sb = consts.tile([P, NPAIR], F32, tag="bk")
        bv_sb = consts.tile([P, NPAIR], F32, tag="bv")
        id_sb = consts.tile([P, P], F16, tag="ident")
        nm_sb = consts.tile([P, P], F16, tag="negmask")
        bm_sb = consts.tile([P, P], F16, tag="binmask")
        sel_sb = consts.tile([65, P], F32R, tag="sel")
        # wq on the sync queue ahead of the x loads so A(0)'s first matmuls
        # start early; everything else one-time goes via the gpsimd SWDGE
        # queue so it streams in parallel with the sync queue.
        nc.sync.dma_start(wq_sb, wqr)
        nc.sync.dma_start(bq_sb, bq)
        dmae = nc.gpsimd if GPSIMD_DMA else nc.sync
        dmae.dma_start(wk_sb, wkr)
        dmae.dma_start(bk_sb, bk)
        dmae.dma_start(wv_sb, wvr)
        dmae.dma_start(bv_sb, bv)
        dmae.dma_start(id_sb, ident)
        dmae.dma_start(nm_sb, negmask)
        dmae.dma_start(bm_sb, binmask)
        dmae.dma_start(sel_sb, sel)
        for qc in range(NQC):
            dmae.dma_start(vaugt[qc][:, :, :, 64:128], onesv)
        dmae.dma_start(wout_sb, woutr)

        xts = {}

        def phase_a_piece(qc, p, xtp, psp):
            """1/4 of chunk qc's projection: qT/kT for pair p, v for st4=p."""
            if qc not in xts:
                cw = slice(qc * 512, (qc + 1) * 512)
                xt = xtp.tile([P, NDT, 512], F16, tag="xt")
                nc.sync.dma_start(xt, xTr[:, :, cw])
                xts[qc] = xt
            xt = xts[qc]
            psq = psp.tile([P, 512], F32, tag="ps", name=f"psq{p}")
            for dt in range(NDT):
                nc.tensor.matmul(psq, wq_sb[:, dt, bass.ts(p, P)],
                                 xt[:, dt],
                                 start=(dt == 0), stop=(dt == NDT - 1),
                                 skip_group_check=SKIP_A)
            nc.vector.tensor_scalar_add(qTt[p][qc], psq, bq_sb[:, p:p + 1])
            psk = psp.tile([P, 512], F32, tag="ps", name=f"psk{p}")
            for dt in range(NDT):
                nc.tensor.matmul(psk, wk_sb[:, dt, bass.ts(p, P)],
                                 xt[:, dt],
                                 start=(dt == 0), stop=(dt == NDT - 1),
                                 skip_group_check=SKIP_A)
            nc.vector.tensor_scalar_add(kTt[p][qc], psk, bk_sb[:, p:p + 1])
            st4 = p
            psv = psp.tile([P, 512], F32, tag="ps", name=f"psv{st4}")
            for dt in range(NDT):
                nc.tensor.matmul(psv, xt[:, dt, bass.ts(st4, P)],
                                 wv_sb[:, dt],
                                 start=(dt == 0), stop=(dt == NDT - 1),
                                 skip_group_check=SKIP_A)
            psv4 = psv.rearrange("q (pp hh d) -> q pp hh d", pp=NPAIR, hh=2)
            nc.vector.tensor_copy(vaugt[qc][:, st4, :, 0:64],
                                  psv4[:, :, 0, :])
            nc.vector.tensor_copy(vaugt[qc][:, st4, :, 128:192],
                                  psv4[:, :, 1, :])

        def phase_b(qc, pp, eTp, qkp, pvp):
            """attention for (chunk qc, pair pp): blocks j=0..4qc+3."""
            cw = slice(qc * 512, (qc + 1) * 512)
            pv0 = pvp.tile([65, 512], F32, tag="pv0", name="pv0")
            pv1 = pvp.tile([P, 512], F32, tag="pv1", name="pv1")
            nblk = 4 * qc + 4
            for j in range(nblk):
                c0 = max(0, (j - 4 * qc) * P)
                diag = j >= 4 * qc
                qk = qkp.tile([P, 2, 512], F32, tag="qk", name="qk")
                # two K=64 QK matmuls back-to-back on PE row-tiles
                # (0,0)/(64,0): they execute concurrently in 64x128 mode.
                mask_on = diag and "m" in B_PARTS and not MASK_DVE
                if "q" in B_PARTS:
                    for h in range(2):
                        hw = slice(64 * h, 64 * h + 64)
                        nc.tensor.matmul(
                            qk[:, h, c0:],
                            kTt[pp][j // 4][hw, (j % 4) * P:(j % 4 + 1) * P],
                            qTt[pp][qc][hw, c0:],
                            start=True, stop=not mask_on,
                            skip_group_check=True)
                if mask_on:
                    # causal mask add (-30000 where sk>sq): one K=128 matmul
                    # per head (two K=64 row-tiles would race on the bank).
                    for h in range(2):
                        nc.tensor.matmul(
                            qk[:, h, c0:c0 + P], id_sb, nm_sb,
                            start=False, stop=True,
                            skip_group_check=True)
                eT = eTp.tile([P, 2, 512], F16, tag="eT", name="eT")
                if "e" in B_PARTS:
                    if EXP_SPAN:
                        nc.scalar.activation(
                            eT[:, :, c0:], qk[:, :, c0:],
                            mybir.ActivationFunctionType.Exp, scale=SCALE)
                    else:
                        for h in range(2):
                            nc.scalar.activation(
                                eT[:, h, c0:], qk[:, h, c0:],
                                mybir.ActivationFunctionType.Exp, scale=SCALE)
                else:
                    nc.vector.memset(eT.bitcast(F32), 1.0)
                if diag and "m" in B_PARTS and MASK_DVE:
                    # zero the causal triangle of eT post-exp on DVE
                    # (keeps the mask off the LDW-serialized PE).
                    for h in range(2):
                        nc.vector.tensor_mul(eT[:, h, c0:c0 + P],
                                             eT[:, h, c0:c0 + P], bm_sb)
                if "p" in B_PARTS:
                    vj = vaugt[j // 4][:, j % 4, pp]
                    nc.tensor.matmul(pv0[:, c0:], vj[:, 0:65], eT[:, 0, c0:],
                                     start=(j == 0), stop=(j == nblk - 1),
                                     skip_group_check=True)
                    nc.tensor.matmul(pv1[:, c0:], vj[:, 64:192], eT[:, 1, c0:],
                                     start=(j == 0), stop=(j == nblk - 1),
                                     skip_group_check=True)
            nc.vector.tensor_copy(outT[0:64, pp, cw], pv0[0:64])
            nc.vector.tensor_copy(outT[64:128, pp, cw], pv1[64:128])
            nc.vector.tensor_copy(rct[pp][64:65, qc], pv0[64:65])
            nc.vector.tensor_copy(rct[pp][0:1, qc], pv1[0:1])

        def phase_c1(qc, pp, qkp, scrp):
            """normalize chunk qc of pair pp by 1/den, add bv."""
            cw = slice(qc * 512, (qc + 1) * 512)
            rc = rct[pp]
            # approx reciprocal (~51 ULP, 5x faster than the iterative
            # divide); fp32 scratch + rounding copy keeps the fp32r
            # verifier happy on the bc-matmul operand.
            if APPROX_RECIP:
                scr = scrp.tile([65, 512], F32, tag="scr", name="scr")
                nc.vector.reciprocal_approx_fast(scr, rc[:, qc].bitcast(F32))
                nc.vector.tensor_copy(rc[:, qc], scr)
            else:
                nc.vector.reciprocal(rc[:, qc], rc[:, qc])
            bc = qkp.tile([P, 2, 512], F32, tag="qk", name="bc")[:, 0]
            nc.tensor.matmul(bc, sel_sb, rc[:, qc], start=True, stop=True)
            nc.vector.tensor_mul(outT[:, pp, cw], outT[:, pp, cw], bc)
            nc.vector.tensor_scalar_add(outT[:, pp, cw], outT[:, pp, cw],
                                        bv_sb[:, pp:pp + 1])

        def phase_c2_piece(st, outp, psp):
            """output projection for one 128-row seq-tile."""
            if True:
                sw = slice(st * P, (st + 1) * P)
                for dc in range(2):
                    ps = psp.tile([P, 512], F32, tag="ps", name="psC")
                    for p in range(NPAIR):
                        nc.tensor.matmul(ps, outT[:, p, sw],
                                         wout_sb[:, p, dc],
                                         start=(p == 0), stop=(p == NPAIR - 1),
                                         skip_group_check=SKIP_A)
                    ot = outp.tile([P, 512], F16, tag="ot")
                    nc.vector.tensor_copy(ot, ps)
                    nc.sync.dma_start(out[sw, dc * 512:(dc + 1) * 512], ot)

        # rc rows 1..63 are read (x0-weighted) by the bc matmul; memset once
        # so the PE never multiplies uninitialized SBUF (they stay ~1.0).
        for pp in range(NPAIR):
            nc.vector.memset(rct[pp].bitcast(F32), 1.0)

        def body(xtp, psp, eTp, qkp, pvp, outp, scrp):
            xts.clear()
            for pp in range(NPAIR):
                phase_a_piece(0, pp, xtp, psp)
            for qc in range(NQC):
                # emission order = scheduler priority: after each attention
                # pair-stream, inline its normalize (C1), the next chunk's
                # projection piece, and the PREVIOUS chunk's out-projection
                # tile, so PE always has fill work during exp latency
                # (pool buffers recycle in emission order).
                for pp in range(NPAIR):
                    if "b" in PHASES:
                        phase_b(qc, pp, eTp, qkp, pvp)
                    if "c" in PHASES:
                        phase_c1(qc, pp, qkp, scrp)
                    if qc + 1 < NQC:
                        phase_a_piece(qc + 1, pp, xtp, psp)
                    if "c" in PHASES and qc >= 1:
                        phase_c2_piece(4 * (qc - 1) + pp, outp, psp)
            if "c" in PHASES:
                for pp in range(NPAIR):
                    phase_c2_piece(12 + pp, outp, psp)

        with tc.tile_pool(name="xtp", bufs=2) as xtp, \
             tc.tile_pool(name="eTp", bufs=3) as eTp, \
             tc.tile_pool(name="scrp", bufs=2) as scrp, \
             tc.tile_pool(name="outp", bufs=3) as outp, \
             tc.tile_pool(name="psp", bufs=2, space="PSUM") as psp, \
             tc.tile_pool(name="qkp", bufs=2, space="PSUM") as qkp, \
             tc.tile_pool(name="pvp", bufs=1, space="PSUM") as pvp:
            if loop == "full":
                with tc.For_i(0, loop_n, 1):
                    body(xtp, psp, eTp, qkp, pvp, outp, scrp)
            else:
                body(xtp, psp, eTp, qkp, pvp, outp, scrp)
    nc.compile()
    _NC_CACHE[key] = nc
    return nc


def prep_core_inputs(x, W_qkv, b_qkv, W_out, core):
    b, g = core // 2, core % 2
    hs = slice(HEADS_LOC * g, HEADS_LOC * (g + 1))
    w3 = W_qkv.reshape(D_MODEL, 3, NHEAD, HEAD_DIM)
    b3 = b_qkv.reshape(3, NHEAD, HEAD_DIM)
    # bc-broadcast selector: den for (chunk, h0) sits at rc row 64,
    # (chunk, h1) at rc row 0; head h covers output partitions [64h, 64h+64).
    sel = np.zeros((65, P), np.float32)
    sel[64, :64] = 1.0
    sel[0, 64:] = 1.0
    tri = np.where(np.arange(P)[:, None] > np.arange(P)[None, :],
                   np.float16(-30000.0), np.float16(0.0))
    bm = np.where(np.arange(P)[:, None] > np.arange(P)[None, :],
                  np.float16(0.0), np.float16(1.0))
    return {
        "xT": np.ascontiguousarray(x[b].T).astype(np.float16),
        "wq": w3[:, 0, hs].reshape(D_MODEL, DL).astype(np.float16),
        "wk": w3[:, 1, hs].reshape(D_MODEL, DL).astype(np.float16),
        "wv": w3[:, 2, hs].reshape(D_MODEL, DL).astype(np.float16),
        "wout": np.ascontiguousarray(
            W_out.reshape(NHEAD, HEAD_DIM, D_MODEL)[hs].reshape(
                DL, D_MODEL)).astype(np.float16),
        "bq": np.ascontiguousarray(b3[0, hs].reshape(NPAIR, P).T,
                                   dtype=np.float32),
        "bk": np.ascontiguousarray(b3[1, hs].reshape(NPAIR, P).T,
                                   dtype=np.float32),
        "bv": np.ascontiguousarray(b3[2, hs].reshape(NPAIR, P).T,
                                   dtype=np.float32),
        "ident": np.eye(P, dtype=np.float16),
        "negmask": np.ascontiguousarray(tri, dtype=np.float16),
        "binmask": np.ascontiguousarray(bm, dtype=np.float16),
        "sel": sel,
        "onesv": np.ones((P, 4, NPAIR, 64), np.float16),
    }


def kernel(x, W_qkv, b_qkv, W_out, b_out):
    x = np.asarray(x, np.float32)
    W_qkv = np.asarray(W_qkv, np.float32)
    b_qkv = np.asarray(b_qkv, np.float32)
    W_out = np.asarray(W_out, np.float32)
    b_out = np.asarray(b_out, np.float32)
    nc = build_nc()
    in_maps = [prep_core_inputs(x, W_qkv, b_qkv, W_out, c)
               for c in range(N_CORES)]
    res = run_bass_kernel_spmd(nc, in_maps, core_ids=list(range(N_CORES)))
    out = np.empty((B, S, D_MODEL), np.float32)
    for b in range(B):
        out[b] = (res.results[2 * b]["out"].astype(np.float32)
                  + res.results[2 * b + 1]["out"].astype(np.float32) + b_out)
    return out
